# revision 1
# baseline (speedup 1.0000x reference)
"""GCNConv kernel for 8x Trainium2 NeuronCores (Bass/Tile).

Reference computation:
    h = x @ W + b                  # [N, 256] @ [256, 128] -> [N, 128]
    out[i] = sum_{e: dst[e]=i} val[e] * h[src[e]]

Strategy (per core; SPMD - one program, per-core data):
  - dst nodes sharded 12500/core (output rows).  Edges partitioned by dst.
  - Bias folded in as a "virtual node": h[N] = b, plus one virtual edge
    (src=N, dst=i, val=sum of vals into i) per dst node.
  - Phase 1: every core computes the full h (fp16) into its own DRAM via
    PE matmuls (host passes x transposed + fp16).
  - Phase 2: per-edge h rows pulled on-chip with dma_gather (int16 indices,
    4 source windows of <=32767 rows).  Per 128-edge chunk, a host-built
    staircase matrix B [128e x 32seg] fp16 (carrying val) is the stationary
    matmul operand -> PSUM partial segment sums.  A second host-built
    one-hot S2 [128seg x 128dst] fp16 matmul accumulates segments into
    per-dst-tile PSUM, which is written out fp32.
All data-dependent structure is padded to the max across cores so the same
program serves all 8 cores.
"""

import sys

for _p in ("/opt/trn_rl_repo",):
    if _p not in sys.path:
        sys.path.insert(0, _p)

import numpy as np

P = 128
MSEG = 32            # segment slots per 128-edge chunk
TPG = 4              # dst tiles (of 128 dst) per processing group
N_CORES = 8


# ---------------------------------------------------------------------------
# Host-side preparation
# ---------------------------------------------------------------------------

def _ceil_to(a, m):
    return -(-a // m) * m


class Plan:
    """Static (core-invariant) program structure + per-core data arrays."""


def build_plan(x, edge_src, edge_dst, edge_vals, weight, bias):
    N, IN_F = x.shape
    OUT_F = weight.shape[1]
    assert N % N_CORES == 0
    ndst = N // N_CORES                    # dst nodes per core
    ndst_pad = _ceil_to(ndst, P)
    ntile = ndst_pad // P                  # dst tiles per core
    ngrp = -(-ntile // TPG)
    hrows = _ceil_to(N + 1, P)             # +1 virtual bias row
    vrow = N                               # virtual node index
    # gather windows: smallest count of equal windows covering hrows with
    # window size <= 32767
    nblk = max(1, -(-hrows // 32512))
    blkr = _ceil_to(-(-hrows // nblk), P)  # rows per window
    nblk = -(-hrows // blkr)
    assert blkr <= 32767

    pl = Plan()
    pl.N, pl.IN_F, pl.OUT_F = N, IN_F, OUT_F
    pl.ndst, pl.ndst_pad, pl.ntile, pl.ngrp = ndst, ndst_pad, ntile, ngrp
    pl.hrows, pl.vrow, pl.nblk, pl.blkr = hrows, vrow, nblk, blkr
    pl.kc = IN_F // P                      # K chunks for projection

    # --- dense inputs ---
    xT = np.zeros((pl.kc, P, hrows), np.float16)
    xT[:, :, :N] = np.ascontiguousarray(x.astype(np.float16).T).reshape(
        pl.kc, P, N
    )
    pl.xT = xT
    pl.W = np.ascontiguousarray(
        weight.astype(np.float16).reshape(pl.kc, P, OUT_F).transpose(1, 0, 2)
    )  # [P, kc, OUT_F]
    pl.bvec = np.ascontiguousarray(bias.astype(np.float16)[None, :])  # [1, OUT_F]

    # --- edges (+ virtual bias edges) ---
    deg_w = np.bincount(edge_dst, weights=edge_vals.astype(np.float64),
                        minlength=N).astype(np.float32)
    src_a = np.concatenate([edge_src.astype(np.int64),
                            np.full(N, vrow, np.int64)])
    dst_a = np.concatenate([edge_dst.astype(np.int64), np.arange(N)])
    val_a = np.concatenate([edge_vals.astype(np.float32), deg_w])

    core = dst_a // ndst
    dl = dst_a % ndst
    tile = dl // P
    grp = tile // TPG
    blk = src_a // blkr
    # stream order per core: (grp, blk, tile, dl)
    order = np.lexsort((dl, tile, blk, grp, core))
    src_a, dst_a, val_a = src_a[order], dst_a[order], val_a[order]
    core, dl, tile, grp, blk = (core[order], dl[order], tile[order],
                                grp[order], blk[order])

    # run = consecutive edges with same (core, tile, blk, dl)
    key_change = np.ones(len(src_a), bool)
    if len(src_a) > 1:
        key_change[1:] = ((core[1:] != core[:-1]) | (blk[1:] != blk[:-1]) |
                          (grp[1:] != grp[:-1]) | (tile[1:] != tile[:-1]) |
                          (dl[1:] != dl[:-1]))
    run_starts = np.nonzero(key_change)[0]
    run_lens = np.diff(np.append(run_starts, len(src_a)))
    r_core = core[run_starts]
    r_grp = grp[run_starts]
    r_blk = blk[run_starts]
    r_tile = tile[run_starts]
    r_dl = dl[run_starts]

    # --- greedy chunk/segment layout, per bucket (core, grp, blk, tile) ---
    # First pass: per-core chunk counts per bucket; second: final layout with
    # static (max-over-core) chunk counts.
    nbuck = ntile * nblk                   # buckets per core (tile-major id)

    def bucket_id(tile_, blk_):
        return tile_ * nblk + blk_

    # greedy chunking of one bucket given run lengths; returns pieces:
    # (run_index, take, chunk, slot) and chunk count
    def greedy(lens):
        pieces = []
        c, s, d = 0, 0, 0
        for ri, ln in enumerate(lens):
            rem = ln
            while rem > 0:
                if s == P or d == MSEG:
                    c += 1
                    s, d = 0, 0
                take = min(P - s, rem)
                pieces.append((ri, take, c, s, d))
                s += take
                d += 1
                rem -= take
        return pieces, (c + 1 if (s > 0 or c == 0) else c)

    # group runs per (core, bucket)
    rb = (r_core * nbuck + bucket_id(r_tile, r_blk)).astype(np.int64)
    rb_order = np.argsort(rb, kind="stable")
    # chunk counts per (core,bucket)
    chunks_cb = np.zeros((N_CORES, nbuck), np.int64)
    bucket_pieces = {}
    i = 0
    rb_sorted = rb[rb_order]
    while i < len(rb_sorted):
        j = i
        while j < len(rb_sorted) and rb_sorted[j] == rb_sorted[i]:
            j += 1
        ridx = rb_order[i:j]
        cb = int(rb_sorted[i])
        pieces, nch = greedy(run_lens[ridx])
        bucket_pieces[cb] = (ridx, pieces)
        chunks_cb[cb // nbuck, cb % nbuck] = nch
        i = j

    chunks_b = chunks_cb.max(axis=0)       # static per-bucket chunk count
    # pad each group's chunk count to a multiple of 16 (PSUM packing) by
    # growing the group's last bucket
    grp_tiles = [list(range(g * TPG, min((g + 1) * TPG, ntile)))
                 for g in range(ngrp)]
    for g in range(ngrp):
        bids = [bucket_id(t, b) for b in range(nblk) for t in grp_tiles[g]]
        tot = int(sum(chunks_b[b] for b in bids))
        pad = _ceil_to(max(tot, 16), 16) - tot
        chunks_b[bucket_id(grp_tiles[g][-1], nblk - 1)] += pad

    # static stream offsets (in chunks) bucket-by-bucket in processing order
    # processing order within group: blk-major, tile-minor
    chunk_off_b = np.zeros(nbuck, np.int64)   # chunk offset of bucket
    order_bids = []
    off = 0
    grp_chunk_off = []
    for g in range(ngrp):
        grp_chunk_off.append(off)
        for b in range(nblk):
            for t in grp_tiles[g]:
                bid = bucket_id(t, b)
                chunk_off_b[bid] = off
                order_bids.append(bid)
                off += int(chunks_b[bid])
    CC = off                                 # total chunks per core
    grp_chunk_off.append(CC)
    TOT = CC * P                             # total edge slots per core

    # gather calls: one per (grp, blk); sizes static
    gather_sizes = np.zeros((ngrp, nblk), np.int64)
    gather_off = np.zeros((ngrp, nblk), np.int64)
    for g in range(ngrp):
        for b in range(nblk):
            sz = sum(int(chunks_b[bucket_id(t, b)]) for t in grp_tiles[g])
            gather_sizes[g, b] = sz * P
    # offsets follow stream order
    acc = 0
    for g in range(ngrp):
        for b in range(nblk):
            gather_off[g, b] = acc
            acc += int(gather_sizes[g, b])
    assert acc == TOT

    # --- fill per-core slot arrays ---
    slot_src = np.zeros((N_CORES, TOT), np.int16)    # idx within window
    Bf = np.zeros((N_CORES, P, CC * MSEG), np.float16)
    # segment bookkeeping for S2
    seg_chunk, seg_slot, seg_dl, seg_core = [], [], [], []
    for cb, (ridx, pieces) in bucket_pieces.items():
        ci, bid = cb // nbuck, cb % nbuck
        base_c = int(chunk_off_b[bid])
        # vectorized expansion
        pr = np.array([p[0] for p in pieces])
        pt = np.array([p[1] for p in pieces])
        pc = np.array([p[2] for p in pieces]) + base_c
        ps_ = np.array([p[3] for p in pieces])
        pd = np.array([p[4] for p in pieces])
        gri = ridx[pr]
        # edge positions: run ri occupies run_starts[ri] .. +run_lens
        # pieces of a run are in order; compute per-piece source start
        # (offset within run): cumsum of takes per run
        src_off = np.zeros(len(pieces), np.int64)
        for k in range(1, len(pieces)):
            if pr[k] == pr[k - 1]:
                src_off[k] = src_off[k - 1] + pt[k - 1]
        e_start = run_starts[gri] + src_off            # into sorted edges
        slot_start = pc * P + ps_                      # into slot arrays
        # expand pieces to edges
        rep = np.repeat(np.arange(len(pieces)), pt)
        within = np.arange(len(rep)) - np.repeat(
            np.concatenate([[0], np.cumsum(pt)[:-1]]), pt)
        e_idx = e_start[rep] + within
        sl_idx = slot_start[rep] + within
        slot_src[ci, sl_idx] = (src_a[e_idx] - blk[e_idx] * blkr).astype(
            np.int16)
        Bf[ci, sl_idx % P, (sl_idx // P) * MSEG + pd[rep]] = (
            val_a[e_idx].astype(np.float16))
        seg_chunk.append(pc)
        seg_slot.append(pd)
        seg_dl.append(r_dl[gri])
        seg_core.append(np.full(len(pieces), ci))

    seg_chunk = np.concatenate(seg_chunk)
    seg_slot = np.concatenate(seg_slot)
    seg_dl = np.concatenate(seg_dl)
    seg_core = np.concatenate(seg_core)

    # --- L2 program (static): chunk -> tile is static via buckets ---
    chunk_tile = np.zeros(CC, np.int64)
    for bid in range(nbuck):
        t = bid // nblk
        c0 = chunk_off_b[bid]
        chunk_tile[c0:c0 + chunks_b[bid]] = t
    # L2 chunk j covers L1 chunks 4j..4j+3; may touch several tiles
    NL2 = CC // 4
    l2_mms = []                 # list of (j, tile) in program order
    for j in range(NL2):
        tiles_here = sorted(set(chunk_tile[4 * j:4 * j + 4].tolist()))
        for t in tiles_here:
            l2_mms.append((j, t))
    NMM = len(l2_mms)
    mm_index = {jt: i for i, jt in enumerate(l2_mms)}
    # start/stop flags per (group, tile)
    mm_start = np.zeros(NMM, bool)
    mm_stop = np.zeros(NMM, bool)
    seen = {}
    for i, (j, t) in enumerate(l2_mms):
        g = next(gg for gg in range(ngrp)
                 if grp_chunk_off[gg] <= 4 * j < grp_chunk_off[gg + 1])
        if (g, t) not in seen:
            mm_start[i] = True
        seen[(g, t)] = i
    for (g, t), i in seen.items():
        mm_stop[i] = True

    # S2 data
    S2f = np.zeros((N_CORES, P, NMM * P), np.float16)
    s_j = seg_chunk // 4
    s_row = (seg_chunk % 4) * MSEG + seg_slot
    s_tile = seg_dl // P
    s_mm = np.array([mm_index[(int(j), int(t))]
                     for j, t in zip(s_j, s_tile)])
    S2f[seg_core, s_row, s_mm * P + (seg_dl % P)] = np.float16(1.0)

    # idx tensor: per gather call, slot q -> [q % 16, off16 + q // 16],
    # replicated 8x across the 128 partitions (one copy per SDMA pair)
    IDX = np.zeros((N_CORES, 16, TOT // 16), np.int16)
    for g in range(ngrp):
        for b in range(nblk):
            o, n = int(gather_off[g, b]), int(gather_sizes[g, b])
            if n == 0:
                continue
            IDX[:, :, o // 16:(o + n) // 16] = slot_src[
                :, o:o + n].reshape(N_CORES, n // 16, 16).transpose(0, 2, 1)
    IDX = np.tile(IDX, (1, 8, 1))          # -> [N_CORES, 128, TOT // 16]

    pl.deg_w = deg_w
    pl.chunks_b, pl.chunk_off_b = chunks_b, chunk_off_b
    pl.grp_tiles, pl.grp_chunk_off = grp_tiles, grp_chunk_off
    pl.CC, pl.TOT, pl.NL2, pl.NMM = CC, TOT, NL2, NMM
    pl.gather_sizes, pl.gather_off = gather_sizes, gather_off
    pl.l2_mms, pl.mm_start, pl.mm_stop = l2_mms, mm_start, mm_stop
    pl.IDX, pl.Bf, pl.S2f = IDX, Bf, S2f
    return pl


# ---------------------------------------------------------------------------
# Device program
# ---------------------------------------------------------------------------

def build_bass(pl):
    import os
    STAGE = int(os.environ.get("K_STAGE", "4"))
    REP1 = int(os.environ.get("K_REP1", "1"))   # phase-1 repeats (timing)
    REP2 = int(os.environ.get("K_REP2", "1"))   # phase-2 repeats (timing)
    import concourse.bass as bass
    import concourse.mybir as mybir
    import concourse.tile as tile
    from concourse import bacc

    f16 = mybir.dt.float16
    f32 = mybir.dt.float32
    i16 = mybir.dt.int16

    NSWQ = int(os.environ.get("K_NSWQ", "4"))
    nc = bacc.Bacc("TRN2", target_bir_lowering=False, debug=False,
                   num_swdge_queues=NSWQ)

    xT_d = nc.dram_tensor("xt", [pl.kc, P, pl.hrows], f16,
                          kind="ExternalInput")
    W_d = nc.dram_tensor("w", [P, pl.kc, pl.OUT_F], f16, kind="ExternalInput")
    b_d = nc.dram_tensor("bvec", [1, pl.OUT_F], f16, kind="ExternalInput")
    idx_d = nc.dram_tensor("idx", [P, pl.TOT // 16], i16,
                           kind="ExternalInput")
    B_d = nc.dram_tensor("bmat", [P, pl.CC * MSEG], f16, kind="ExternalInput")
    S2_d = nc.dram_tensor("s2", [P, pl.NMM * P], f16, kind="ExternalInput")
    out_d = nc.dram_tensor("out", [pl.ndst_pad, pl.OUT_F], f32,
                           kind="ExternalOutput")
    h_d = nc.dram_tensor("hbuf", [pl.hrows, pl.OUT_F], f16)

    OF = pl.OUT_F
    RB = 512                       # projection row-batch
    n_rb = pl.hrows // RB if pl.hrows % RB == 0 else pl.hrows // RB + 1

    with tile.TileContext(nc) as tc:
        # ---------------- Phase 1: h = x @ W ----------------
        with (
            tc.tile_pool(name="pconst", bufs=1) as pconst,
            tc.tile_pool(name="pxt", bufs=3) as pxt,
            tc.tile_pool(name="phs", bufs=3) as phs,
            tc.tile_pool(name="ppsum", bufs=2, space="PSUM") as ppsum,
        ):
            W_sb = pconst.tile([P, pl.kc, OF], f16)
            nc.sync.dma_start(W_sb[:], W_d[:])
            b_sb = pconst.tile([1, OF], f16)
            nc.sync.dma_start(b_sb[:], b_d[:])

            for _rep1 in range(REP1):
                for j in range(n_rb):
                    r0 = j * RB
                    nrows = min(RB, pl.hrows - r0)
                    nch = nrows // P
                    xt = pxt.tile([P, pl.kc, RB], f16, tag="xt")
                    nc.sync.dma_start(
                        xt[:, :, :nrows],
                        xT_d[:, :, r0:r0 + nrows].rearrange("k p c -> p k c"),
                    )
                    ps = ppsum.tile([P, RB], f32, tag="pj")
                    for rc in range(nch):
                        for k in range(pl.kc):
                            nc.tensor.matmul(
                                ps[:, rc * P:(rc + 1) * P],
                                lhsT=xt[:, k, rc * P:(rc + 1) * P],
                                rhs=W_sb[:, k, :],
                                start=(k == 0),
                                stop=(k == pl.kc - 1),
                            )
                    hs = phs.tile([P, RB], f16, tag="hs")
                    nc.vector.tensor_copy(hs[:, :nrows], ps[:, :nrows])
                    nc.sync.dma_start(
                        h_d[r0:r0 + nrows, :].rearrange("(c p) f -> p c f", p=P),
                        hs[:, :nrows].rearrange("p (c f) -> p c f", f=OF),
                    )
            # virtual bias row - written after the projection loop so the
            # projection's batch covering this row cannot clobber it
            nc.sync.dma_start(h_d[pl.vrow:pl.vrow + 1, :], b_sb[:])

        # ---------------- Phase 2: gather + L1 + L2 ----------------
        with (
            tc.tile_pool(name="pidx", bufs=2) as pidx,
            tc.tile_pool(name="pmsg", bufs=2) as pmsg,
            tc.tile_pool(name="pB", bufs=3) as pB,
            tc.tile_pool(name="pS2", bufs=3) as pS2,
            tc.tile_pool(name="pP", bufs=3) as pP,
            tc.tile_pool(name="pout", bufs=2) as pout,
            tc.tile_pool(name="psL1", bufs=2, space="PSUM") as psL1,
            tc.tile_pool(name="psL2", bufs=TPG, space="PSUM") as psL2,
        ):
            for _rep2 in range(REP2):
                mm_i = 0          # global L2 mm counter
                for g in range(pl.ngrp):
                    tiles_g = pl.grp_tiles[g]
                    c_lo, c_hi = pl.grp_chunk_off[g], pl.grp_chunk_off[g + 1]
                    nch_g = c_hi - c_lo
                    # gathers for this group
                    msgs = {}
                    for b in range(pl.nblk):
                        if STAGE < 1:
                            continue
                        n = int(pl.gather_sizes[g, b])
                        if n == 0:
                            continue
                        o = int(pl.gather_off[g, b])
                        ixt = pidx.tile([P, n // 16], i16, tag="idx")
                        nc.sync.dma_start(ixt[:], idx_d[:, o // 16:(o + n) // 16])
                        if STAGE < 2:
                            continue
                        mt = pmsg.tile([P, n // P, OF], f16, tag="msg")
                        r0 = b * pl.blkr
                        r1 = min(r0 + pl.blkr, pl.hrows)
                        GMAX = int(os.environ.get("K_GMAX", "8192"))
                        for q0 in range(0, n, GMAX):
                            qn = min(GMAX, n - q0)
                            nc.gpsimd.dma_gather(
                                out_ap=mt[:, q0 // P:(q0 + qn) // P, :],
                                in_ap=h_d[r0:r1, :],
                                idxs_ap=ixt[:, q0 // 16:(q0 + qn) // 16],
                                num_idxs=qn,
                                num_idxs_reg=qn,
                                elem_size=OF,
                                single_packet=False,
                            )
                        msgs[b] = (mt, o)

                    l2ps = {}
                    for t in tiles_g:
                        l2ps[t] = psL2.tile([P, OF], f32, tag="l2",
                                            name=f"l2ps_{g}_{t}")

                    # B tiles per psum-group of 16 chunks
                    n_pg = nch_g // 16 if STAGE >= 3 else 0
                    for pg in range(n_pg):
                        c0 = c_lo + pg * 16
                        Bt = pB.tile([P, 16 * MSEG], f16, tag="B")
                        nc.sync.dma_start(
                            Bt[:], B_d[:, c0 * MSEG:(c0 + 16) * MSEG])
                        ps1 = psL1.tile([P, 16 * MSEG], f32, tag="ps1")
                        for cc in range(16):
                            c = c0 + cc
                            # locate the msgs tile holding this chunk
                            so = c * P
                            b = None
                            for bb in range(pl.nblk):
                                o = int(pl.gather_off[g, bb])
                                n = int(pl.gather_sizes[g, bb])
                                if o <= so < o + n:
                                    b = bb
                                    break
                            mt, o = msgs[b]
                            ci = (so - o) // P
                            cg = cc % 4
                            w = (cc // 4) % 4
                            nc.tensor.matmul(
                                ps1[32 * cg:32 * (cg + 1), w * P:(w + 1) * P],
                                lhsT=Bt[:, cc * MSEG:(cc + 1) * MSEG],
                                rhs=mt[:, ci, :],
                                start=True,
                                stop=True,
                                tile_position=(0, 32 * cg),
                            )
                        Pt = pP.tile([P, 4, OF], f16, tag="P")
                        nc.vector.tensor_copy(
                            Pt[:].rearrange("p a b -> p (a b)"), ps1[:])
                        # L2 mms for the 4 L2-chunks of this psum-group
                        j0 = (c0 // 4)
                        mms_here = []
                        while mm_i < pl.NMM and pl.l2_mms[mm_i][0] < j0 + 4:
                            mms_here.append(mm_i)
                            mm_i += 1
                        if mms_here and STAGE >= 4:
                            s2t = pS2.tile([P, len(mms_here) * P], f16, tag="s2",
                                           name=f"s2_{g}_{pg}")
                            nc.sync.dma_start(
                                s2t[:],
                                S2_d[:, mms_here[0] * P:
                                     (mms_here[-1] + 1) * P])
                            for q, mi in enumerate(mms_here):
                                j, t = pl.l2_mms[mi]
                                nc.tensor.matmul(
                                    l2ps[t][:],
                                    lhsT=s2t[:, q * P:(q + 1) * P],
                                    rhs=Pt[:, j % 4, :],
                                    start=bool(pl.mm_start[mi]),
                                    stop=bool(pl.mm_stop[mi]),
                                )
                    # evict group's dst tiles
                    for t in tiles_g:
                        if STAGE < 1:
                            continue
                        ot = pout.tile([P, OF], f32, tag="out")
                        if STAGE >= 4:
                            nc.vector.tensor_copy(ot[:], l2ps[t][:])
                        else:
                            nc.gpsimd.memset(ot[:], 0.0)
                        nc.sync.dma_start(out_d[t * P:(t + 1) * P, :], ot[:])

    nc.compile()
    return nc


# ---------------------------------------------------------------------------
# Entry point
# ---------------------------------------------------------------------------

def kernel(x, edge_src, edge_dst, edge_vals, weight, bias,
           _want_trace=False, _n_cores=None):
    x = np.asarray(x)
    edge_src = np.asarray(edge_src)
    edge_dst = np.asarray(edge_dst)
    edge_vals = np.asarray(edge_vals)
    weight = np.asarray(weight)
    bias = np.asarray(bias)

    pl = build_plan(x, edge_src, edge_dst, edge_vals, weight, bias)
    nc = build_bass(pl)

    from concourse.bass_utils import run_bass_kernel_spmd

    ncores = N_CORES if _n_cores is None else _n_cores
    in_maps = []
    for ci in range(ncores):
        in_maps.append({
            "xt": pl.xT,
            "w": pl.W,
            "bvec": pl.bvec,
            "idx": np.ascontiguousarray(pl.IDX[ci]),
            "bmat": np.ascontiguousarray(pl.Bf[ci]),
            "s2": np.ascontiguousarray(pl.S2f[ci]),
        })
    res = run_bass_kernel_spmd(nc, in_maps, core_ids=list(range(ncores)),
                               trace=_want_trace)
    outs = [res.results[ci]["out"][:pl.ndst, :] for ci in range(ncores)]
    if ncores < N_CORES:
        outs += [np.zeros((pl.ndst, pl.OUT_F), np.float32)] * (N_CORES - ncores)
    full = np.concatenate(outs, axis=0).astype(np.float32)
    if _want_trace:
        kernel._last_results = res
    return full



# revision 3
# speedup vs baseline: 1.1048x; 1.1048x over previous
"""GCNConv kernel for 8x Trainium2 NeuronCores (Bass/Tile).

Reference computation:
    h = x @ W + b                  # [N, 256] @ [256, 128] -> [N, 128]
    out[i] = sum_{e: dst[e]=i} val[e] * h[src[e]]

Strategy (per core; SPMD - one program, per-core data):
  - dst nodes sharded 12500/core (output rows).  Edges partitioned by dst.
  - Bias folded in as a "virtual node": h[N] = b, plus one virtual edge
    (src=N, dst=i, val=sum of vals into i) per dst node.
  - Phase 1: every core computes the full h (fp16) into its own DRAM via
    PE matmuls (host passes x transposed + fp16).
  - Phase 2: per-edge h rows pulled on-chip with dma_gather (int16 indices,
    4 source windows of <=32767 rows).  Per 128-edge chunk, a host-built
    staircase matrix B [128e x 32seg] fp16 (carrying val) is the stationary
    matmul operand -> PSUM partial segment sums.  A second host-built
    one-hot S2 [128seg x 128dst] fp16 matmul accumulates segments into
    per-dst-tile PSUM, which is written out fp32.
All data-dependent structure is padded to the max across cores so the same
program serves all 8 cores.
"""

import sys

for _p in ("/opt/trn_rl_repo",):
    if _p not in sys.path:
        sys.path.insert(0, _p)

import numpy as np

P = 128
MSEG = 32            # segment slots per 128-edge chunk
TPG = 4              # dst tiles (of 128 dst) per processing group
N_CORES = 8


# ---------------------------------------------------------------------------
# Host-side preparation
# ---------------------------------------------------------------------------

def _ceil_to(a, m):
    return -(-a // m) * m


class Plan:
    """Static (core-invariant) program structure + per-core data arrays."""


def build_plan(x, edge_src, edge_dst, edge_vals, weight, bias):
    N, IN_F = x.shape
    OUT_F = weight.shape[1]
    assert N % N_CORES == 0
    ndst = N // N_CORES                    # dst nodes per core
    ndst_pad = _ceil_to(ndst, P)
    ntile = ndst_pad // P                  # dst tiles per core
    ngrp = -(-ntile // TPG)
    hrows = _ceil_to(N + 1, P)             # +1 virtual bias row
    vrow = N                               # virtual node index
    # gather windows: smallest count of equal windows covering hrows with
    # window size <= 32767
    nblk = max(1, -(-hrows // 32512))
    blkr = _ceil_to(-(-hrows // nblk), P)  # rows per window
    nblk = -(-hrows // blkr)
    assert blkr <= 32767

    pl = Plan()
    pl.N, pl.IN_F, pl.OUT_F = N, IN_F, OUT_F
    pl.ndst, pl.ndst_pad, pl.ntile, pl.ngrp = ndst, ndst_pad, ntile, ngrp
    pl.hrows, pl.vrow, pl.nblk, pl.blkr = hrows, vrow, nblk, blkr
    pl.kc = IN_F // P                      # K chunks for projection

    # --- dense inputs ---
    xT = np.zeros((pl.kc, P, hrows), np.float16)
    xT[:, :, :N] = np.ascontiguousarray(x.astype(np.float16).T).reshape(
        pl.kc, P, N
    )
    pl.xT = xT
    pl.W = np.ascontiguousarray(
        weight.astype(np.float16).reshape(pl.kc, P, OUT_F).transpose(1, 0, 2)
    )  # [P, kc, OUT_F]
    pl.bvec = np.ascontiguousarray(bias.astype(np.float16)[None, :])  # [1, OUT_F]

    # --- edges (+ virtual bias edges) ---
    deg_w = np.bincount(edge_dst, weights=edge_vals.astype(np.float64),
                        minlength=N).astype(np.float32)
    src_a = np.concatenate([edge_src.astype(np.int64),
                            np.full(N, vrow, np.int64)])
    dst_a = np.concatenate([edge_dst.astype(np.int64), np.arange(N)])
    val_a = np.concatenate([edge_vals.astype(np.float32), deg_w])

    core = dst_a // ndst
    dl = dst_a % ndst
    tile = dl // P
    grp = tile // TPG
    blk = src_a // blkr
    # stream order per core: (grp, blk, tile, dl)
    order = np.lexsort((dl, tile, blk, grp, core))
    src_a, dst_a, val_a = src_a[order], dst_a[order], val_a[order]
    core, dl, tile, grp, blk = (core[order], dl[order], tile[order],
                                grp[order], blk[order])

    # run = consecutive edges with same (core, tile, blk, dl)
    key_change = np.ones(len(src_a), bool)
    if len(src_a) > 1:
        key_change[1:] = ((core[1:] != core[:-1]) | (blk[1:] != blk[:-1]) |
                          (grp[1:] != grp[:-1]) | (tile[1:] != tile[:-1]) |
                          (dl[1:] != dl[:-1]))
    run_starts = np.nonzero(key_change)[0]
    run_lens = np.diff(np.append(run_starts, len(src_a)))
    r_core = core[run_starts]
    r_grp = grp[run_starts]
    r_blk = blk[run_starts]
    r_tile = tile[run_starts]
    r_dl = dl[run_starts]

    # --- greedy chunk/segment layout, per bucket (core, grp, blk, tile) ---
    # First pass: per-core chunk counts per bucket; second: final layout with
    # static (max-over-core) chunk counts.
    nbuck = ntile * nblk                   # buckets per core (tile-major id)

    def bucket_id(tile_, blk_):
        return tile_ * nblk + blk_

    # greedy chunking of one bucket given run lengths; returns pieces:
    # (run_index, take, chunk, slot) and chunk count
    def greedy(lens):
        pieces = []
        c, s, d = 0, 0, 0
        for ri, ln in enumerate(lens):
            rem = ln
            while rem > 0:
                if s == P or d == MSEG:
                    c += 1
                    s, d = 0, 0
                take = min(P - s, rem)
                pieces.append((ri, take, c, s, d))
                s += take
                d += 1
                rem -= take
        return pieces, (c + 1 if (s > 0 or c == 0) else c)

    # group runs per (core, bucket)
    rb = (r_core * nbuck + bucket_id(r_tile, r_blk)).astype(np.int64)
    rb_order = np.argsort(rb, kind="stable")
    # chunk counts per (core,bucket)
    chunks_cb = np.zeros((N_CORES, nbuck), np.int64)
    bucket_pieces = {}
    i = 0
    rb_sorted = rb[rb_order]
    while i < len(rb_sorted):
        j = i
        while j < len(rb_sorted) and rb_sorted[j] == rb_sorted[i]:
            j += 1
        ridx = rb_order[i:j]
        cb = int(rb_sorted[i])
        pieces, nch = greedy(run_lens[ridx])
        bucket_pieces[cb] = (ridx, pieces)
        chunks_cb[cb // nbuck, cb % nbuck] = nch
        i = j

    chunks_b = chunks_cb.max(axis=0)       # static per-bucket chunk count
    # pad each group's chunk count to a multiple of 16 (PSUM packing) by
    # growing the group's last bucket
    grp_tiles = [list(range(g * TPG, min((g + 1) * TPG, ntile)))
                 for g in range(ngrp)]
    for g in range(ngrp):
        bids = [bucket_id(t, b) for b in range(nblk) for t in grp_tiles[g]]
        tot = int(sum(chunks_b[b] for b in bids))
        pad = _ceil_to(max(tot, 16), 16) - tot
        chunks_b[bucket_id(grp_tiles[g][-1], nblk - 1)] += pad

    # static stream offsets (in chunks) bucket-by-bucket in processing order
    # processing order within group: blk-major, tile-minor
    chunk_off_b = np.zeros(nbuck, np.int64)   # chunk offset of bucket
    order_bids = []
    off = 0
    grp_chunk_off = []
    for g in range(ngrp):
        grp_chunk_off.append(off)
        for b in range(nblk):
            for t in grp_tiles[g]:
                bid = bucket_id(t, b)
                chunk_off_b[bid] = off
                order_bids.append(bid)
                off += int(chunks_b[bid])
    CC = off                                 # total chunks per core
    grp_chunk_off.append(CC)
    TOT = CC * P                             # total edge slots per core

    # gather calls: one per (grp, blk); sizes static
    gather_sizes = np.zeros((ngrp, nblk), np.int64)
    gather_off = np.zeros((ngrp, nblk), np.int64)
    for g in range(ngrp):
        for b in range(nblk):
            sz = sum(int(chunks_b[bucket_id(t, b)]) for t in grp_tiles[g])
            gather_sizes[g, b] = sz * P
    # offsets follow stream order
    acc = 0
    for g in range(ngrp):
        for b in range(nblk):
            gather_off[g, b] = acc
            acc += int(gather_sizes[g, b])
    assert acc == TOT

    # --- fill per-core slot arrays ---
    slot_src = np.zeros((N_CORES, TOT), np.int16)    # idx within window
    Bf = np.zeros((N_CORES, P, CC * MSEG), np.float16)
    # segment bookkeeping for S2
    seg_chunk, seg_slot, seg_dl, seg_core = [], [], [], []
    for cb, (ridx, pieces) in bucket_pieces.items():
        ci, bid = cb // nbuck, cb % nbuck
        base_c = int(chunk_off_b[bid])
        # vectorized expansion
        pr = np.array([p[0] for p in pieces])
        pt = np.array([p[1] for p in pieces])
        pc = np.array([p[2] for p in pieces]) + base_c
        ps_ = np.array([p[3] for p in pieces])
        pd = np.array([p[4] for p in pieces])
        gri = ridx[pr]
        # edge positions: run ri occupies run_starts[ri] .. +run_lens
        # pieces of a run are in order; compute per-piece source start
        # (offset within run): cumsum of takes per run
        src_off = np.zeros(len(pieces), np.int64)
        for k in range(1, len(pieces)):
            if pr[k] == pr[k - 1]:
                src_off[k] = src_off[k - 1] + pt[k - 1]
        e_start = run_starts[gri] + src_off            # into sorted edges
        slot_start = pc * P + ps_                      # into slot arrays
        # expand pieces to edges
        rep = np.repeat(np.arange(len(pieces)), pt)
        within = np.arange(len(rep)) - np.repeat(
            np.concatenate([[0], np.cumsum(pt)[:-1]]), pt)
        e_idx = e_start[rep] + within
        sl_idx = slot_start[rep] + within
        slot_src[ci, sl_idx] = (src_a[e_idx] - blk[e_idx] * blkr).astype(
            np.int16)
        Bf[ci, sl_idx % P, (sl_idx // P) * MSEG + pd[rep]] = (
            val_a[e_idx].astype(np.float16))
        seg_chunk.append(pc)
        seg_slot.append(pd)
        seg_dl.append(r_dl[gri])
        seg_core.append(np.full(len(pieces), ci))

    seg_chunk = np.concatenate(seg_chunk)
    seg_slot = np.concatenate(seg_slot)
    seg_dl = np.concatenate(seg_dl)
    seg_core = np.concatenate(seg_core)

    # --- L2 program (static): chunk -> tile is static via buckets ---
    chunk_tile = np.zeros(CC, np.int64)
    for bid in range(nbuck):
        t = bid // nblk
        c0 = chunk_off_b[bid]
        chunk_tile[c0:c0 + chunks_b[bid]] = t
    # L2 chunk j covers L1 chunks 4j..4j+3; may touch several tiles
    NL2 = CC // 4
    l2_mms = []                 # list of (j, tile) in program order
    for j in range(NL2):
        tiles_here = sorted(set(chunk_tile[4 * j:4 * j + 4].tolist()))
        for t in tiles_here:
            l2_mms.append((j, t))
    NMM = len(l2_mms)
    mm_index = {jt: i for i, jt in enumerate(l2_mms)}
    # start/stop flags per (group, tile)
    mm_start = np.zeros(NMM, bool)
    mm_stop = np.zeros(NMM, bool)
    seen = {}
    for i, (j, t) in enumerate(l2_mms):
        g = next(gg for gg in range(ngrp)
                 if grp_chunk_off[gg] <= 4 * j < grp_chunk_off[gg + 1])
        if (g, t) not in seen:
            mm_start[i] = True
        seen[(g, t)] = i
    for (g, t), i in seen.items():
        mm_stop[i] = True

    # S2 data
    S2f = np.zeros((N_CORES, P, NMM * P), np.float16)
    s_j = seg_chunk // 4
    s_row = (seg_chunk % 4) * MSEG + seg_slot
    s_tile = seg_dl // P
    s_mm = np.array([mm_index[(int(j), int(t))]
                     for j, t in zip(s_j, s_tile)])
    S2f[seg_core, s_row, s_mm * P + (seg_dl % P)] = np.float16(1.0)

    # idx tensor: per gather call, slot q -> [q % 16, off16 + q // 16],
    # replicated 8x across the 128 partitions (one copy per SDMA pair)
    IDX = np.zeros((N_CORES, 16, TOT // 16), np.int16)
    for g in range(ngrp):
        for b in range(nblk):
            o, n = int(gather_off[g, b]), int(gather_sizes[g, b])
            if n == 0:
                continue
            IDX[:, :, o // 16:(o + n) // 16] = slot_src[
                :, o:o + n].reshape(N_CORES, n // 16, 16).transpose(0, 2, 1)
    IDX = np.tile(IDX, (1, 8, 1))          # -> [N_CORES, 128, TOT // 16]

    pl.deg_w = deg_w
    pl.chunks_b, pl.chunk_off_b = chunks_b, chunk_off_b
    pl.grp_tiles, pl.grp_chunk_off = grp_tiles, grp_chunk_off
    pl.CC, pl.TOT, pl.NL2, pl.NMM = CC, TOT, NL2, NMM
    pl.gather_sizes, pl.gather_off = gather_sizes, gather_off
    pl.l2_mms, pl.mm_start, pl.mm_stop = l2_mms, mm_start, mm_stop
    pl.IDX, pl.Bf, pl.S2f = IDX, Bf, S2f
    return pl


# ---------------------------------------------------------------------------
# Device program
# ---------------------------------------------------------------------------

def build_bass(pl):
    import os
    STAGE = int(os.environ.get("K_STAGE", "4"))
    REP1 = int(os.environ.get("K_REP1", "1"))   # phase-1 repeats (timing)
    REP2 = int(os.environ.get("K_REP2", "1"))   # phase-2 repeats (timing)
    import concourse.bass as bass
    import concourse.mybir as mybir
    import concourse.tile as tile
    from concourse import bacc

    f16 = mybir.dt.float16
    f32 = mybir.dt.float32
    i16 = mybir.dt.int16

    NSWQ = int(os.environ.get("K_NSWQ", "4"))
    nc = bacc.Bacc("TRN2", target_bir_lowering=False, debug=False,
                   num_swdge_queues=NSWQ)

    xT_d = nc.dram_tensor("xt", [pl.kc, P, pl.hrows], f16,
                          kind="ExternalInput")
    W_d = nc.dram_tensor("w", [P, pl.kc, pl.OUT_F], f16, kind="ExternalInput")
    b_d = nc.dram_tensor("bvec", [1, pl.OUT_F], f16, kind="ExternalInput")
    idx_d = nc.dram_tensor("idx", [P, pl.TOT // 16], i16,
                           kind="ExternalInput")
    B_d = nc.dram_tensor("bmat", [P, pl.CC * MSEG], f16, kind="ExternalInput")
    S2_d = nc.dram_tensor("s2", [P, pl.NMM * P], f16, kind="ExternalInput")
    out_d = nc.dram_tensor("out", [pl.ndst_pad, pl.OUT_F], f32,
                           kind="ExternalOutput")
    h_d = nc.dram_tensor("hbuf", [pl.hrows, pl.OUT_F], f16)

    OF = pl.OUT_F
    RB = 512                       # projection row-batch
    n_rb = pl.hrows // RB if pl.hrows % RB == 0 else pl.hrows // RB + 1

    with tile.TileContext(nc) as tc:
        # ---------------- Phase 1: h = x @ W ----------------
        with (
            tc.tile_pool(name="pconst", bufs=1) as pconst,
            tc.tile_pool(name="pxt", bufs=3) as pxt,
            tc.tile_pool(name="phs", bufs=3) as phs,
            tc.tile_pool(name="ppsum", bufs=2, space="PSUM") as ppsum,
        ):
            W_sb = pconst.tile([P, pl.kc, OF], f16)
            nc.sync.dma_start(W_sb[:], W_d[:])
            b_sb = pconst.tile([1, OF], f16)
            nc.sync.dma_start(b_sb[:], b_d[:])

            for _rep1 in range(REP1):
                for j in range(n_rb):
                    r0 = j * RB
                    nrows = min(RB, pl.hrows - r0)
                    nch = nrows // P
                    xt = pxt.tile([P, pl.kc, RB], f16, tag="xt")
                    nc.sync.dma_start(
                        xt[:, :, :nrows],
                        xT_d[:, :, r0:r0 + nrows].rearrange("k p c -> p k c"),
                    )
                    ps = ppsum.tile([P, RB], f32, tag="pj")
                    for rc in range(nch):
                        for k in range(pl.kc):
                            nc.tensor.matmul(
                                ps[:, rc * P:(rc + 1) * P],
                                lhsT=xt[:, k, rc * P:(rc + 1) * P],
                                rhs=W_sb[:, k, :],
                                start=(k == 0),
                                stop=(k == pl.kc - 1),
                            )
                    hs = phs.tile([P, RB], f16, tag="hs")
                    nc.vector.tensor_copy(hs[:, :nrows], ps[:, :nrows])
                    nc.sync.dma_start(
                        h_d[r0:r0 + nrows, :].rearrange("(c p) f -> p c f", p=P),
                        hs[:, :nrows].rearrange("p (c f) -> p c f", f=OF),
                    )
            # virtual bias row - written after the projection loop so the
            # projection's batch covering this row cannot clobber it
            nc.sync.dma_start(h_d[pl.vrow:pl.vrow + 1, :], b_sb[:])

        # ---------------- Phase 2: gather + L1 + L2 ----------------
        with (
            tc.tile_pool(name="pidx", bufs=2) as pidx,
            tc.tile_pool(name="pmsg", bufs=2) as pmsg,
            tc.tile_pool(name="pB", bufs=3) as pB,
            tc.tile_pool(name="pS2", bufs=3) as pS2,
            tc.tile_pool(name="pP", bufs=3) as pP,
            tc.tile_pool(name="pout", bufs=2) as pout,
            tc.tile_pool(name="psL1", bufs=2, space="PSUM") as psL1,
            tc.tile_pool(name="psL2", bufs=TPG, space="PSUM") as psL2,
        ):
            _gq = [0]             # gather queue rotation counter
            for _rep2 in range(REP2):
                mm_i = 0          # global L2 mm counter
                for g in range(pl.ngrp):
                    tiles_g = pl.grp_tiles[g]
                    c_lo, c_hi = pl.grp_chunk_off[g], pl.grp_chunk_off[g + 1]
                    nch_g = c_hi - c_lo
                    # gathers for this group
                    msgs = {}
                    for b in range(pl.nblk):
                        if STAGE < 1:
                            continue
                        n = int(pl.gather_sizes[g, b])
                        if n == 0:
                            continue
                        o = int(pl.gather_off[g, b])
                        ixt = pidx.tile([P, n // 16], i16, tag="idx")
                        nc.sync.dma_start(ixt[:], idx_d[:, o // 16:(o + n) // 16])
                        if STAGE < 2:
                            continue
                        mt = pmsg.tile([P, n // P, OF], f16, tag="msg")
                        r0 = b * pl.blkr
                        r1 = min(r0 + pl.blkr, pl.hrows)
                        GMAX = int(os.environ.get("K_GMAX", "8192"))
                        for q0 in range(0, n, GMAX):
                            qn = min(GMAX, n - q0)
                            nc.gpsimd.dma_gather(
                                out_ap=mt[:, q0 // P:(q0 + qn) // P, :],
                                in_ap=h_d[r0:r1, :],
                                idxs_ap=ixt[:, q0 // 16:(q0 + qn) // 16],
                                num_idxs=qn,
                                num_idxs_reg=qn,
                                elem_size=OF,
                                single_packet=False,
                                queue_num=_gq[0] % NSWQ,
                            )
                            _gq[0] += 1
                        msgs[b] = (mt, o)

                    l2ps = {}
                    for t in tiles_g:
                        l2ps[t] = psL2.tile([P, OF], f32, tag="l2",
                                            name=f"l2ps_{g}_{t}")

                    # B tiles per psum-group of 16 chunks
                    n_pg = nch_g // 16 if STAGE >= 3 else 0
                    for pg in range(n_pg):
                        c0 = c_lo + pg * 16
                        Bt = pB.tile([P, 16 * MSEG], f16, tag="B")
                        nc.sync.dma_start(
                            Bt[:], B_d[:, c0 * MSEG:(c0 + 16) * MSEG])
                        ps1 = psL1.tile([P, 16 * MSEG], f32, tag="ps1")
                        for cc in range(16):
                            c = c0 + cc
                            # locate the msgs tile holding this chunk
                            so = c * P
                            b = None
                            for bb in range(pl.nblk):
                                o = int(pl.gather_off[g, bb])
                                n = int(pl.gather_sizes[g, bb])
                                if o <= so < o + n:
                                    b = bb
                                    break
                            mt, o = msgs[b]
                            ci = (so - o) // P
                            cg = cc % 4
                            w = (cc // 4) % 4
                            nc.tensor.matmul(
                                ps1[32 * cg:32 * (cg + 1), w * P:(w + 1) * P],
                                lhsT=Bt[:, cc * MSEG:(cc + 1) * MSEG],
                                rhs=mt[:, ci, :],
                                start=True,
                                stop=True,
                                tile_position=(0, 32 * cg),
                            )
                        Pt = pP.tile([P, 4, OF], f16, tag="P")
                        nc.vector.tensor_copy(
                            Pt[:].rearrange("p a b -> p (a b)"), ps1[:])
                        # L2 mms for the 4 L2-chunks of this psum-group
                        j0 = (c0 // 4)
                        mms_here = []
                        while mm_i < pl.NMM and pl.l2_mms[mm_i][0] < j0 + 4:
                            mms_here.append(mm_i)
                            mm_i += 1
                        if mms_here and STAGE >= 4:
                            s2t = pS2.tile([P, len(mms_here) * P], f16, tag="s2",
                                           name=f"s2_{g}_{pg}")
                            nc.sync.dma_start(
                                s2t[:],
                                S2_d[:, mms_here[0] * P:
                                     (mms_here[-1] + 1) * P])
                            for q, mi in enumerate(mms_here):
                                j, t = pl.l2_mms[mi]
                                nc.tensor.matmul(
                                    l2ps[t][:],
                                    lhsT=s2t[:, q * P:(q + 1) * P],
                                    rhs=Pt[:, j % 4, :],
                                    start=bool(pl.mm_start[mi]),
                                    stop=bool(pl.mm_stop[mi]),
                                )
                    # evict group's dst tiles
                    for t in tiles_g:
                        if STAGE < 1:
                            continue
                        ot = pout.tile([P, OF], f32, tag="out")
                        if STAGE >= 4:
                            nc.vector.tensor_copy(ot[:], l2ps[t][:])
                        else:
                            nc.gpsimd.memset(ot[:], 0.0)
                        nc.sync.dma_start(out_d[t * P:(t + 1) * P, :], ot[:])

    nc.compile()
    return nc


# ---------------------------------------------------------------------------
# Entry point
# ---------------------------------------------------------------------------

def kernel(x, edge_src, edge_dst, edge_vals, weight, bias,
           _want_trace=False, _n_cores=None):
    x = np.asarray(x)
    edge_src = np.asarray(edge_src)
    edge_dst = np.asarray(edge_dst)
    edge_vals = np.asarray(edge_vals)
    weight = np.asarray(weight)
    bias = np.asarray(bias)

    pl = build_plan(x, edge_src, edge_dst, edge_vals, weight, bias)
    nc = build_bass(pl)

    from concourse.bass_utils import run_bass_kernel_spmd

    ncores = N_CORES if _n_cores is None else _n_cores
    in_maps = []
    for ci in range(ncores):
        in_maps.append({
            "xt": pl.xT,
            "w": pl.W,
            "bvec": pl.bvec,
            "idx": np.ascontiguousarray(pl.IDX[ci]),
            "bmat": np.ascontiguousarray(pl.Bf[ci]),
            "s2": np.ascontiguousarray(pl.S2f[ci]),
        })
    res = run_bass_kernel_spmd(nc, in_maps, core_ids=list(range(ncores)),
                               trace=_want_trace)
    outs = [res.results[ci]["out"][:pl.ndst, :] for ci in range(ncores)]
    if ncores < N_CORES:
        outs += [np.zeros((pl.ndst, pl.OUT_F), np.float32)] * (N_CORES - ncores)
    full = np.concatenate(outs, axis=0).astype(np.float32)
    if _want_trace:
        kernel._last_results = res
    return full



# revision 4
# speedup vs baseline: 1.1366x; 1.0288x over previous
"""GCNConv kernel for 8x Trainium2 NeuronCores (Bass/Tile) — v2.

Reference computation:
    h = x @ W + b                  # [N, 256] @ [256, 128] -> [N, 128]
    out[i] = sum_{e: dst[e]=i} val[e] * h[src[e]]

Strategy (per core; SPMD — one program, per-core data):
  - dst nodes sharded 12500/core (output rows); edges partitioned by dst.
  - src rows split into 4 windows of <=32512 rows (int16 gather indices).
  - Processing is window-sweep-outer: for each window w, all 98 dst tiles
    are processed using only h rows of window w; per-tile partial sums
    accumulate in a persistent SBUF f32 accumulator across sweeps.
  - Per (window, tile): edges sorted by src, padded to 128-slot chunks
    (chunk count = max across cores, so one program serves all 8 cores).
    Per chunk: per-edge h rows pulled on-chip with dma_gather (256B rows;
    gathers rotate across the 4 SWDGE queues so all four Q7 core-pairs
    generate descriptors concurrently — 4x the single-queue rate), a
    [128 slot x 128 dst] one-hot-times-val matrix B is generated on-device
    (DVE tensor_scalar: (iota == dlt[p]) * val[p]), and one PE matmul
    accumulates B.T @ msgs into the tile's PSUM.
  - Bias is folded in per tile as a rank-1 matmul: deg[t].T @ bias, where
    deg[i] = sum of edge_vals into dst i (host-computed).
  - Phase 1 (h = x @ W) is pipelined: window w+1's projection runs
    interleaved with sweep w's gathers/matmuls; h windows are separate
    DRAM tensors so the Tile framework serializes only true RAW deps.
"""

import sys

for _p in ("/opt/trn_rl_repo",):
    if _p not in sys.path:
        sys.path.insert(0, _p)

import numpy as np

P = 128
N_CORES = 8
WROWS = 32512          # gather window rows (int16 index limit)
RB = 512               # phase-1 row batch


def _ceil_to(a, m):
    return -(-a // m) * m


class Plan:
    pass


def build_plan(x, edge_src, edge_dst, edge_vals, weight, bias):
    N, IN_F = x.shape
    OUT_F = weight.shape[1]
    assert N % N_CORES == 0
    ndst = N // N_CORES
    ndst_pad = _ceil_to(ndst, P)
    ntile = ndst_pad // P

    NW = -(-N // WROWS)
    wstart = [w * WROWS for w in range(NW)]
    wrows = [min(WROWS, N - s) for s in wstart]
    wrows_pad = [_ceil_to(r, P) for r in wrows]
    hrows_pad = _ceil_to(N, P)          # xT column padding (>= sum windows)
    xt_cols = max(hrows_pad, wstart[-1] + wrows_pad[-1])

    pl = Plan()
    pl.N, pl.IN_F, pl.OUT_F = N, IN_F, OUT_F
    pl.ndst, pl.ndst_pad, pl.ntile = ndst, ndst_pad, ntile
    pl.NW, pl.wstart, pl.wrows, pl.wrows_pad = NW, wstart, wrows, wrows_pad
    pl.kc = IN_F // P
    pl.xt_cols = xt_cols

    # --- dense inputs ---
    xT = np.zeros((pl.kc, P, xt_cols), np.float16)
    xT[:, :, :N] = np.ascontiguousarray(x.astype(np.float16).T).reshape(
        pl.kc, P, N)
    pl.xT = xT
    pl.W = np.ascontiguousarray(
        weight.astype(np.float16).reshape(pl.kc, P, OUT_F).transpose(1, 0, 2))
    pl.bvec = np.ascontiguousarray(bias.astype(np.float16)[None, :])

    # --- per-dst weighted degree (for the bias rank-1 update) ---
    deg_w = np.bincount(edge_dst, weights=edge_vals.astype(np.float64),
                        minlength=N).astype(np.float32)
    degv = np.zeros((N_CORES, 1, ndst_pad), np.float16)
    degv[:, 0, :ndst] = deg_w.reshape(N_CORES, ndst)
    pl.degv = degv

    # --- edge partitioning ---
    src = edge_src.astype(np.int64)
    dst = edge_dst.astype(np.int64)
    val = edge_vals.astype(np.float32)
    core = dst // ndst
    dl = dst % ndst
    tile = dl // P
    w = src // WROWS

    order = np.lexsort((src, tile, w, core))
    src, dl, tile, w, core, val = (src[order], dl[order], tile[order],
                                   w[order], core[order], val[order])

    # edge counts per (core, w, tile)
    cnt = np.zeros((N_CORES, NW, ntile), np.int64)
    np.add.at(cnt, (core, w, tile), 1)
    cmax = cnt.max(axis=0)                       # [NW, ntile]
    nchunk_wt = np.maximum(1, -(-cmax // P))     # >=1 chunk per (w, t)
    pl.nchunk_wt = nchunk_wt
    NCHUNK = int(nchunk_wt.sum())
    TOT = NCHUNK * P
    pl.NCHUNK, pl.TOT = NCHUNK, TOT

    # chunk offsets in processing order (w-major, tile-minor)
    chunk_off = np.zeros((NW, ntile), np.int64)
    off = 0
    wchunk0 = []                                  # first chunk of window w
    for ww in range(NW):
        wchunk0.append(off)
        for t in range(ntile):
            chunk_off[ww, t] = off
            off += int(nchunk_wt[ww, t])
    wchunk0.append(off)
    pl.chunk_off, pl.wchunk0 = chunk_off, wchunk0

    # static chunk program: per chunk -> (w, tile, start, stop)
    chunk_w = np.zeros(NCHUNK, np.int64)
    chunk_t = np.zeros(NCHUNK, np.int64)
    chunk_start = np.zeros(NCHUNK, bool)
    chunk_stop = np.zeros(NCHUNK, bool)
    for ww in range(NW):
        for t in range(ntile):
            c0 = int(chunk_off[ww, t])
            n = int(nchunk_wt[ww, t])
            chunk_w[c0:c0 + n] = ww
            chunk_t[c0:c0 + n] = t
            chunk_start[c0] = True
            chunk_stop[c0 + n - 1] = True
    pl.chunk_w, pl.chunk_t = chunk_w, chunk_t
    pl.chunk_start, pl.chunk_stop = chunk_start, chunk_stop

    # --- per-core slot arrays ---
    # slot position: edges of (core, w, t) go to slots
    # [chunk_off[w,t]*P + k for k in range(cnt)]
    ecount = cnt[core, w, tile]                  # not used; need rank in group
    # rank of each edge within its (core, w, t) group (edges are sorted)
    grp = (core * NW + w) * ntile + tile
    changes = np.ones(len(grp), bool)
    changes[1:] = grp[1:] != grp[:-1]
    gstart = np.nonzero(changes)[0]
    rank = np.arange(len(grp)) - np.repeat(gstart, np.diff(
        np.append(gstart, len(grp))))

    slot = chunk_off[w, tile] * P + rank         # per-edge slot (per-core)

    IDXW = np.zeros((N_CORES, TOT), np.int16)
    dltv = np.full((N_CORES, TOT), -1.0, np.float32)
    valv = np.zeros((N_CORES, TOT), np.float32)
    IDXW[core, slot] = (src - np.array(wstart)[w]).astype(np.int16)
    dltv[core, slot] = (dl % P).astype(np.float32)
    valv[core, slot] = val

    # reshape slot arrays to [P, NCHUNK] (partition = slot % P)
    pl.dlt = np.ascontiguousarray(
        dltv.reshape(N_CORES, NCHUNK, P).transpose(0, 2, 1))
    pl.val = np.ascontiguousarray(
        valv.reshape(N_CORES, NCHUNK, P).transpose(0, 2, 1))

    # gather calls: per window, GMAX-slot pieces; idx tensor wrapped %16
    import os
    GMAX = int(os.environ.get("K_GMAX", "8192"))
    calls = []                                   # (w, slot_off, nslots)
    for ww in range(NW):
        s0, s1 = wchunk0[ww] * P, wchunk0[ww + 1] * P
        for o in range(s0, s1, GMAX):
            calls.append((ww, o, min(GMAX, s1 - o)))
    pl.calls = calls

    IDX = np.zeros((N_CORES, 16, TOT // 16), np.int16)
    for (ww, o, n) in calls:
        IDX[:, :, o // 16:(o + n) // 16] = IDXW[:, o:o + n].reshape(
            N_CORES, n // 16, 16).transpose(0, 2, 1)
    pl.IDX = np.tile(IDX, (1, 8, 1))             # [N_CORES, 128, TOT//16]

    # iota constant [P, P]: row j has value j in each partition
    pl.iota = np.ascontiguousarray(
        np.broadcast_to(np.arange(P, dtype=np.float16)[None, :], (P, P)))
    return pl


# ---------------------------------------------------------------------------
# Device program
# ---------------------------------------------------------------------------

def build_bass(pl):
    import os
    import concourse.mybir as mybir
    import concourse.tile as tile
    from concourse import bacc

    f16 = mybir.dt.float16
    f32 = mybir.dt.float32
    i16 = mybir.dt.int16

    NSWQ = int(os.environ.get("K_NSWQ", "4"))
    P1SPREAD = int(os.environ.get("K_P1SPREAD", "4"))
    nc = bacc.Bacc("TRN2", target_bir_lowering=False, debug=False,
                   num_swdge_queues=NSWQ)

    OF = pl.OUT_F
    xT_d = nc.dram_tensor("xt", [pl.kc, P, pl.xt_cols], f16,
                          kind="ExternalInput")
    W_d = nc.dram_tensor("w", [P, pl.kc, OF], f16, kind="ExternalInput")
    b_d = nc.dram_tensor("bvec", [1, OF], f16, kind="ExternalInput")
    deg_d = nc.dram_tensor("degv", [1, pl.ndst_pad], f16,
                           kind="ExternalInput")
    iota_d = nc.dram_tensor("iota", [P, P], f16, kind="ExternalInput")
    idx_d = nc.dram_tensor("idx", [P, pl.TOT // 16], i16,
                           kind="ExternalInput")
    dlt_d = nc.dram_tensor("dlt", [P, pl.NCHUNK], f32, kind="ExternalInput")
    val_d = nc.dram_tensor("val", [P, pl.NCHUNK], f32, kind="ExternalInput")
    out_d = nc.dram_tensor("out", [pl.ndst_pad, OF], f32,
                           kind="ExternalOutput")
    h_ds = [nc.dram_tensor(f"hbuf{w}", [pl.wrows_pad[w], OF], f16)
            for w in range(pl.NW)]

    with tile.TileContext(nc) as tc:
        with (
            tc.tile_pool(name="pconst", bufs=1) as pconst,
            tc.tile_pool(name="pacc", bufs=1) as pacc,
            tc.tile_pool(name="pxt", bufs=4) as pxt,
            tc.tile_pool(name="phs", bufs=4) as phs,
            tc.tile_pool(name="pp1", bufs=2, space="PSUM") as pp1,
            tc.tile_pool(name="pidx", bufs=3) as pidx,
            tc.tile_pool(name="pmsg", bufs=3) as pmsg,
            tc.tile_pool(name="pB", bufs=6) as pB,
            tc.tile_pool(name="psL1", bufs=4, space="PSUM") as psL1,
        ):
            W_sb = pconst.tile([P, pl.kc, OF], f16)
            nc.sync.dma_start(W_sb[:], W_d[:])
            b_sb = pconst.tile([1, OF], f16)
            nc.sync.dma_start(b_sb[:], b_d[:])
            deg_sb = pconst.tile([1, pl.ndst_pad], f16)
            nc.sync.dma_start(deg_sb[:], deg_d[:])
            iota_sb = pconst.tile([P, P], f16)
            nc.sync.dma_start(iota_sb[:], iota_d[:])
            dlt_sb = pconst.tile([P, pl.NCHUNK], f32)
            nc.sync.dma_start(dlt_sb[:], dlt_d[:])
            val_sb = pconst.tile([P, pl.NCHUNK], f32)
            nc.sync.dma_start(val_sb[:], val_d[:])
            acc = pacc.tile([P, pl.ntile, OF], f32)

            # ---------------- phase-1 batch generator ----------------
            def phase1_batches(w):
                nrows_w = pl.wrows_pad[w]
                for r0 in range(0, nrows_w, RB):
                    nrows = min(RB, nrows_w - r0)
                    abs0 = pl.wstart[w] + r0
                    nch = nrows // P
                    xt = pxt.tile([P, pl.kc, RB], f16, tag="xt")
                    nc.sync.dma_start(
                        xt[:, :, :nrows],
                        xT_d[:, :, abs0:abs0 + nrows].rearrange(
                            "k p c -> p k c"),
                    )
                    ps = pp1.tile([P, RB], f32, tag="pj")
                    for rc in range(nch):
                        for k in range(pl.kc):
                            nc.tensor.matmul(
                                ps[:, rc * P:(rc + 1) * P],
                                lhsT=xt[:, k, rc * P:(rc + 1) * P],
                                rhs=W_sb[:, k, :],
                                start=(k == 0),
                                stop=(k == pl.kc - 1),
                            )
                    hs = phs.tile([P, RB], f16, tag="hs")
                    nc.scalar.activation(
                        hs[:, :nrows], ps[:, :nrows],
                        mybir.ActivationFunctionType.Copy)
                    nc.sync.dma_start(
                        h_ds[w][r0:r0 + nrows, :].rearrange(
                            "(c p) f -> p c f", p=P),
                        hs[:, :nrows].rearrange("p (c f) -> p c f", f=OF),
                    )
                    yield

            # ---------------- main pipeline ----------------
            gens = [phase1_batches(w) for w in range(pl.NW)]

            def drain(g, k=None):
                i = 0
                for _ in g:
                    i += 1
                    if k is not None and i >= k:
                        return

            drain(gens[0])                       # h window 0 fully projected

            gq = 0                               # gather queue rotation
            psum = {}                            # tile -> psum tile
            for ci, (w, o, n) in enumerate(pl.calls):
                # interleave next window's projection
                if w + 1 < pl.NW:
                    drain(gens[w + 1], P1SPREAD)
                ixt = pidx.tile([P, n // 16], i16, tag="idx")
                nc.sync.dma_start(ixt[:], idx_d[:, o // 16:(o + n) // 16])
                mt = pmsg.tile([P, n // P, OF], f16, tag="msg")
                nc.gpsimd.dma_gather(
                    out_ap=mt[:],
                    in_ap=h_ds[w][:, :],
                    idxs_ap=ixt[:],
                    num_idxs=n,
                    num_idxs_reg=n,
                    elem_size=OF,
                    single_packet=False,
                    queue_num=gq % NSWQ,
                )
                gq += 1
                c0, c1 = o // P, (o + n) // P
                for c in range(c0, c1):
                    t = int(pl.chunk_t[c])
                    if pl.chunk_start[c]:
                        psum[t] = psL1.tile([P, OF], f32, tag="l1",
                                            name=f"ps_{w}_{t}")
                    Bt = pB.tile([P, P], f16, tag="B")
                    nc.vector.tensor_scalar(
                        Bt[:], iota_sb[:], dlt_sb[:, c:c + 1],
                        val_sb[:, c:c + 1],
                        mybir.AluOpType.is_equal, mybir.AluOpType.mult)
                    last = bool(pl.chunk_stop[c])
                    fin = last and w == pl.NW - 1
                    nc.tensor.matmul(
                        psum[t][:],
                        lhsT=Bt[:],
                        rhs=mt[:, c - c0, :],
                        start=bool(pl.chunk_start[c]),
                        stop=last and not fin,
                    )
                    if fin:                      # bias rank-1, then evict
                        nc.tensor.matmul(
                            psum[t][:],
                            lhsT=deg_sb[:, t * P:(t + 1) * P],
                            rhs=b_sb[:],
                            start=False,
                            stop=True,
                        )
                    if last:
                        if w == 0:
                            nc.scalar.activation(
                                acc[:, t, :], psum[t][:],
                                mybir.ActivationFunctionType.Copy)
                        else:
                            nc.vector.tensor_tensor(
                                acc[:, t, :], acc[:, t, :], psum[t][:],
                                mybir.AluOpType.add)
                        if fin:
                            nc.sync.dma_start(out_d[t * P:(t + 1) * P, :],
                                              acc[:, t, :])
                        del psum[t]

    nc.compile()
    return nc


# ---------------------------------------------------------------------------
# Entry point
# ---------------------------------------------------------------------------

def kernel(x, edge_src, edge_dst, edge_vals, weight, bias,
           _want_trace=False, _n_cores=None):
    x = np.asarray(x)
    edge_src = np.asarray(edge_src)
    edge_dst = np.asarray(edge_dst)
    edge_vals = np.asarray(edge_vals)
    weight = np.asarray(weight)
    bias = np.asarray(bias)

    pl = build_plan(x, edge_src, edge_dst, edge_vals, weight, bias)
    nc = build_bass(pl)

    from concourse.bass_utils import run_bass_kernel_spmd

    ncores = N_CORES if _n_cores is None else _n_cores
    in_maps = []
    for ci in range(ncores):
        in_maps.append({
            "xt": pl.xT,
            "w": pl.W,
            "bvec": pl.bvec,
            "degv": np.ascontiguousarray(pl.degv[ci]),
            "iota": pl.iota,
            "idx": np.ascontiguousarray(pl.IDX[ci]),
            "dlt": np.ascontiguousarray(pl.dlt[ci]),
            "val": np.ascontiguousarray(pl.val[ci]),
        })
    res = run_bass_kernel_spmd(nc, in_maps, core_ids=list(range(ncores)),
                               trace=_want_trace)
    outs = [res.results[ci]["out"][:pl.ndst, :] for ci in range(ncores)]
    if ncores < N_CORES:
        outs += [np.zeros((pl.ndst, pl.OUT_F), np.float32)] * (
            N_CORES - ncores)
    full = np.concatenate(outs, axis=0).astype(np.float32)
    if _want_trace:
        kernel._last_results = res
    return full


# revision 12
# speedup vs baseline: 1.8108x; 1.5932x over previous
"""GCNConv kernel for 8x Trainium2 NeuronCores (Bass/Tile) — v2.

Reference computation:
    h = x @ W + b                  # [N, 256] @ [256, 128] -> [N, 128]
    out[i] = sum_{e: dst[e]=i} val[e] * h[src[e]]

Strategy (per core; SPMD — one program, per-core data):
  - dst nodes sharded 12500/core (output rows); edges partitioned by dst.
  - src rows split into 4 windows of <=32512 rows (int16 gather indices).
  - Processing is window-sweep-outer: for each window w, all 98 dst tiles
    are processed using only h rows of window w; per-tile partial sums
    accumulate in a persistent SBUF f32 accumulator across sweeps.
  - Per (window, tile): edges sorted by src, padded to 128-slot chunks
    (chunk count = max across cores, so one program serves all 8 cores).
    Per chunk: per-edge h rows pulled on-chip with dma_gather (256B rows;
    gathers rotate across the 4 SWDGE queues so all four Q7 core-pairs
    generate descriptors concurrently — 4x the single-queue rate), a
    [128 slot x 128 dst] one-hot-times-val matrix B is generated on-device
    (DVE tensor_scalar: (iota == dlt[p]) * val[p]), and one PE matmul
    accumulates B.T @ msgs into the tile's PSUM.
  - Bias is folded in per tile as a rank-1 matmul: deg[t].T @ bias, where
    deg[i] = sum of edge_vals into dst i (host-computed).
  - Phase 1 (h = x @ W) is pipelined: window w+1's projection runs
    interleaved with sweep w's gathers/matmuls; h windows are separate
    DRAM tensors so the Tile framework serializes only true RAW deps.
"""

import sys

for _p in ("/opt/trn_rl_repo",):
    if _p not in sys.path:
        sys.path.insert(0, _p)

import numpy as np

P = 128
N_CORES = 8
WROWS = 32512          # gather window rows (int16 index limit)
RB = 512               # phase-1 row batch


def _ceil_to(a, m):
    return -(-a // m) * m


class Plan:
    pass


def build_plan(x, edge_src, edge_dst, edge_vals, weight, bias):
    N, IN_F = x.shape
    OUT_F = weight.shape[1]
    assert N % N_CORES == 0
    ndst = N // N_CORES
    ndst_pad = _ceil_to(ndst, P)
    ntile = ndst_pad // P

    NW = -(-N // WROWS)
    wstart = [w * WROWS for w in range(NW)]
    wrows = [min(WROWS, N - s) for s in wstart]
    wrows_pad = [_ceil_to(r, P) for r in wrows]
    hrows_pad = _ceil_to(N, P)          # xT column padding (>= sum windows)
    xt_cols = max(hrows_pad, wstart[-1] + wrows_pad[-1])

    pl = Plan()
    pl.N, pl.IN_F, pl.OUT_F = N, IN_F, OUT_F
    pl.ndst, pl.ndst_pad, pl.ntile = ndst, ndst_pad, ntile
    pl.NW, pl.wstart, pl.wrows, pl.wrows_pad = NW, wstart, wrows, wrows_pad
    pl.kc = IN_F // P
    pl.xt_cols = xt_cols

    # --- dense inputs ---
    xT = np.zeros((pl.kc, P, xt_cols), np.float16)
    xT[:, :, :N] = np.ascontiguousarray(x.astype(np.float16).T).reshape(
        pl.kc, P, N)
    pl.xT = xT
    pl.W = np.ascontiguousarray(
        weight.astype(np.float16).reshape(pl.kc, P, OUT_F).transpose(1, 0, 2))
    pl.bvec = np.ascontiguousarray(bias.astype(np.float16)[None, :])

    # --- per-dst weighted degree (for the bias rank-1 update) ---
    deg_w = np.bincount(edge_dst, weights=edge_vals.astype(np.float64),
                        minlength=N).astype(np.float32)
    degv = np.zeros((N_CORES, 1, ndst_pad), np.float16)
    degv[:, 0, :ndst] = deg_w.reshape(N_CORES, ndst)
    pl.degv = degv

    # --- edge partitioning ---
    src = edge_src.astype(np.int64)
    dst = edge_dst.astype(np.int64)
    val = edge_vals.astype(np.float32)
    core = dst // ndst
    dl = dst % ndst
    tile = dl // P
    w = src // WROWS

    order = np.lexsort((src, tile, w, core))
    src, dl, tile, w, core, val = (src[order], dl[order], tile[order],
                                   w[order], core[order], val[order])

    # edge counts per (core, w, tile)
    cnt = np.zeros((N_CORES, NW, ntile), np.int64)
    np.add.at(cnt, (core, w, tile), 1)
    cmax = cnt.max(axis=0)                       # [NW, ntile]
    nchunk_wt = np.maximum(1, -(-cmax // P))     # >=1 chunk per (w, t)
    pl.nchunk_wt = nchunk_wt
    NCHUNK = int(nchunk_wt.sum())
    TOT = NCHUNK * P
    pl.NCHUNK, pl.TOT = NCHUNK, TOT

    # chunk offsets in processing order (w-major, tile-minor)
    chunk_off = np.zeros((NW, ntile), np.int64)
    off = 0
    wchunk0 = []                                  # first chunk of window w
    for ww in range(NW):
        wchunk0.append(off)
        for t in range(ntile):
            chunk_off[ww, t] = off
            off += int(nchunk_wt[ww, t])
    wchunk0.append(off)
    pl.chunk_off, pl.wchunk0 = chunk_off, wchunk0

    # static chunk program: per chunk -> (w, tile, start, stop)
    chunk_w = np.zeros(NCHUNK, np.int64)
    chunk_t = np.zeros(NCHUNK, np.int64)
    chunk_start = np.zeros(NCHUNK, bool)
    chunk_stop = np.zeros(NCHUNK, bool)
    for ww in range(NW):
        for t in range(ntile):
            c0 = int(chunk_off[ww, t])
            n = int(nchunk_wt[ww, t])
            chunk_w[c0:c0 + n] = ww
            chunk_t[c0:c0 + n] = t
            chunk_start[c0] = True
            chunk_stop[c0 + n - 1] = True
    pl.chunk_w, pl.chunk_t = chunk_w, chunk_t
    pl.chunk_start, pl.chunk_stop = chunk_start, chunk_stop

    # --- per-core slot arrays ---
    # slot position: edges of (core, w, t) go to slots
    # [chunk_off[w,t]*P + k for k in range(cnt)]
    ecount = cnt[core, w, tile]                  # not used; need rank in group
    # rank of each edge within its (core, w, t) group (edges are sorted)
    grp = (core * NW + w) * ntile + tile
    changes = np.ones(len(grp), bool)
    changes[1:] = grp[1:] != grp[:-1]
    gstart = np.nonzero(changes)[0]
    rank = np.arange(len(grp)) - np.repeat(gstart, np.diff(
        np.append(gstart, len(grp))))

    slot = chunk_off[w, tile] * P + rank         # per-edge slot (per-core)

    IDXW = np.zeros((N_CORES, TOT), np.int16)
    dltv = np.full((N_CORES, TOT), -1.0, np.float32)
    valv = np.zeros((N_CORES, TOT), np.float32)
    IDXW[core, slot] = (src - np.array(wstart)[w]).astype(np.int16)
    dltv[core, slot] = (dl % P).astype(np.float32)
    valv[core, slot] = val

    # reshape slot arrays to [P, NCHUNK] (partition = slot % P)
    pl.dlt = np.ascontiguousarray(
        dltv.reshape(N_CORES, NCHUNK, P).transpose(0, 2, 1)).astype(np.float16)
    pl.val = np.ascontiguousarray(
        valv.reshape(N_CORES, NCHUNK, P).transpose(0, 2, 1)).astype(np.float16)

    # gather calls: per window, GMAX-slot pieces; idx tensor wrapped %16
    import os
    GMAX = int(os.environ.get("K_GMAX", "8192"))
    calls = []                                   # (w, slot_off, nslots)
    for ww in range(NW):
        s0, s1 = wchunk0[ww] * P, wchunk0[ww + 1] * P
        for o in range(s0, s1, GMAX):
            calls.append((ww, o, min(GMAX, s1 - o)))
    pl.calls = calls

    IDX = np.zeros((N_CORES, 16, TOT // 16), np.int16)
    for (ww, o, n) in calls:
        IDX[:, :, o // 16:(o + n) // 16] = IDXW[:, o:o + n].reshape(
            N_CORES, n // 16, 16).transpose(0, 2, 1)
    pl.IDX = np.tile(IDX, (1, 8, 1))             # [N_CORES, 128, TOT//16]

    # iota constant [P, P]: row j has value j in each partition
    pl.iota = np.ascontiguousarray(
        np.broadcast_to(np.arange(P, dtype=np.float16)[None, :], (P, P)))
    return pl


# ---------------------------------------------------------------------------
# Device program
# ---------------------------------------------------------------------------

def build_bass(pl):
    import os
    import concourse.bass as bass
    import concourse.mybir as mybir
    import concourse.tile as tile
    from concourse import bacc

    f16 = mybir.dt.float16
    f32 = mybir.dt.float32
    i16 = mybir.dt.int16

    NSWQ = int(os.environ.get("K_NSWQ", "4"))
    P1SPREAD = int(os.environ.get("K_P1SPREAD", "5"))
    BGB = int(os.environ.get("K_BGB", "16"))
    nc = bacc.Bacc("TRN2", target_bir_lowering=False, debug=False,
                   num_swdge_queues=NSWQ)

    OF = pl.OUT_F
    xT_d = nc.dram_tensor("xt", [pl.kc, P, pl.xt_cols], f16,
                          kind="ExternalInput")
    W_d = nc.dram_tensor("w", [P, pl.kc, OF], f16, kind="ExternalInput")
    b_d = nc.dram_tensor("bvec", [1, OF], f16, kind="ExternalInput")
    deg_d = nc.dram_tensor("degv", [1, pl.ndst_pad], f16,
                           kind="ExternalInput")
    iota_d = nc.dram_tensor("iota", [P, P], f16, kind="ExternalInput")
    idx_d = nc.dram_tensor("idx", [P, pl.TOT // 16], i16,
                           kind="ExternalInput")
    dlt_d = nc.dram_tensor("dlt", [P, pl.NCHUNK], f16, kind="ExternalInput")
    val_d = nc.dram_tensor("val", [P, pl.NCHUNK], f16, kind="ExternalInput")
    out_d = nc.dram_tensor("out", [pl.ndst_pad, OF], f32,
                           kind="ExternalOutput")
    h_ds = [nc.dram_tensor(f"hbuf{w}", [pl.wrows_pad[w], OF], f16)
            for w in range(pl.NW)]

    with tile.TileContext(nc) as tc:
        with (
            tc.tile_pool(name="pconst", bufs=1) as pconst,
            tc.tile_pool(name="pacc", bufs=1) as pacc,
            tc.tile_pool(name="pxt", bufs=4) as pxt,
            tc.tile_pool(name="phs", bufs=4) as phs,
            tc.tile_pool(name="pp1", bufs=2, space="PSUM") as pp1,
            tc.tile_pool(name="pidx", bufs=3) as pidx,
            tc.tile_pool(name="pmsg", bufs=4) as pmsg,
            tc.tile_pool(name="pB", bufs=6) as pB,
            tc.tile_pool(name="psL1", bufs=4, space="PSUM") as psL1,
        ):
            W_sb = pconst.tile([P, pl.kc, OF], f16)
            nc.sync.dma_start(W_sb[:], W_d[:])
            b_sb = pconst.tile([1, OF], f16)
            nc.sync.dma_start(b_sb[:], b_d[:])
            deg_sb = pconst.tile([1, pl.ndst_pad], f16)
            nc.sync.dma_start(deg_sb[:], deg_d[:])
            iota_sb = pconst.tile([P, P], f16)
            nc.sync.dma_start(iota_sb[:], iota_d[:])
            dlt_sb = pconst.tile([P, pl.NCHUNK], f16)
            nc.sync.dma_start(dlt_sb[:], dlt_d[:])
            val_sb = pconst.tile([P, pl.NCHUNK], f16)
            nc.sync.dma_start(val_sb[:], val_d[:])
            acc = pacc.tile([P, pl.ntile, OF], f32)

            # ---------------- phase-1 batch generator ----------------
            def phase1_batches(w):
                nrows_w = pl.wrows_pad[w]
                for r0 in range(0, nrows_w, RB):
                    nrows = min(RB, nrows_w - r0)
                    abs0 = pl.wstart[w] + r0
                    nch = nrows // P
                    xt = pxt.tile([P, pl.kc, RB], f16, tag="xt")
                    nc.sync.dma_start(
                        xt[:, :, :nrows],
                        xT_d[:, :, abs0:abs0 + nrows].rearrange(
                            "k p c -> p k c"),
                    )
                    ps = pp1.tile([P, RB], f32, tag="pj")
                    for rc in range(nch):
                        for k in range(pl.kc):
                            nc.tensor.matmul(
                                ps[:, rc * P:(rc + 1) * P],
                                lhsT=xt[:, k, rc * P:(rc + 1) * P],
                                rhs=W_sb[:, k, :],
                                start=(k == 0),
                                stop=(k == pl.kc - 1),
                            )
                    hs = phs.tile([P, RB], f16, tag="hs")
                    nc.scalar.activation(
                        hs[:, :nrows], ps[:, :nrows],
                        mybir.ActivationFunctionType.Copy)
                    nc.sync.dma_start(
                        h_ds[w][r0:r0 + nrows, :].rearrange(
                            "(c p) f -> p c f", p=P),
                        hs[:, :nrows].rearrange("p (c f) -> p c f", f=OF),
                    )
                    yield

            # ---------------- main pipeline ----------------
            gens = [phase1_batches(w) for w in range(pl.NW)]

            def drain(g, k=None):
                i = 0
                for _ in g:
                    i += 1
                    if k is not None and i >= k:
                        return

            drain(gens[0])                       # h window 0 fully projected

            gq = 0                               # gather queue rotation
            psum = {}                            # tile -> psum tile
            for ci, (w, o, n) in enumerate(pl.calls):
                # interleave next window's projection
                if w + 1 < pl.NW:
                    drain(gens[w + 1], P1SPREAD)
                ixt = pidx.tile([P, n // 16], i16, tag="idx")
                nc.sync.dma_start(ixt[:], idx_d[:, o // 16:(o + n) // 16])
                mt = pmsg.tile([P, n // P, OF], f16, tag="msg")
                nc.gpsimd.dma_gather(
                    out_ap=mt[:],
                    in_ap=h_ds[w][:, :],
                    idxs_ap=ixt[:],
                    num_idxs=n,
                    num_idxs_reg=n,
                    elem_size=OF,
                    single_packet=False,
                    queue_num=gq % NSWQ,
                )
                gq += 1
                c0, c1 = o // P, (o + n) // P
                Bt = None
                for c in range(c0, c1):
                    t = int(pl.chunk_t[c])
                    if pl.chunk_start[c]:
                        psum[t] = psL1.tile([P, OF], f32, tag="l1",
                                            name=f"ps_{w}_{t}")
                    if (c - c0) % BGB == 0:
                        # batched B generation: k chunks per DVE op pair
                        b0 = c
                        k = min(BGB, c1 - c)
                        Bt = pB.tile([P, BGB, P], f16, tag="B")
                        ia = iota_sb[:]
                        da = dlt_sb[:, b0:b0 + k]
                        va = val_sb[:, b0:b0 + k]
                        iota_b = bass.AP(
                            ia.tensor, ia.offset,
                            [list(ia.ap[0]), [0, k], list(ia.ap[1])])
                        dlt_b = bass.AP(
                            da.tensor, da.offset,
                            [list(da.ap[0]), list(da.ap[1]), [0, P]])
                        val_b = bass.AP(
                            va.tensor, va.offset,
                            [list(va.ap[0]), list(va.ap[1]), [0, P]])
                        nc.vector.tensor_tensor(
                            Bt[:, :k, :], iota_b, dlt_b,
                            mybir.AluOpType.is_equal)
                        nc.vector.tensor_tensor(
                            Bt[:, :k, :], Bt[:, :k, :], val_b,
                            mybir.AluOpType.mult)
                    last = bool(pl.chunk_stop[c])
                    fin = last and w == pl.NW - 1
                    nc.tensor.matmul(
                        psum[t][:],
                        lhsT=Bt[:, (c - b0), :],
                        rhs=mt[:, c - c0, :],
                        start=bool(pl.chunk_start[c]),
                        stop=last and not fin,
                    )
                    if fin:                      # bias rank-1, then evict
                        nc.tensor.matmul(
                            psum[t][:],
                            lhsT=deg_sb[:, t * P:(t + 1) * P],
                            rhs=b_sb[:],
                            start=False,
                            stop=True,
                        )
                    if last:
                        if w == 0:
                            nc.scalar.activation(
                                acc[:, t, :], psum[t][:],
                                mybir.ActivationFunctionType.Copy)
                        else:
                            nc.vector.tensor_tensor(
                                acc[:, t, :], acc[:, t, :], psum[t][:],
                                mybir.AluOpType.add)
                        if fin:
                            nc.sync.dma_start(out_d[t * P:(t + 1) * P, :],
                                              acc[:, t, :])
                        del psum[t]

    nc.compile()
    return nc


# ---------------------------------------------------------------------------
# Entry point
# ---------------------------------------------------------------------------

def kernel(x, edge_src, edge_dst, edge_vals, weight, bias,
           _want_trace=False, _n_cores=None):
    x = np.asarray(x)
    edge_src = np.asarray(edge_src)
    edge_dst = np.asarray(edge_dst)
    edge_vals = np.asarray(edge_vals)
    weight = np.asarray(weight)
    bias = np.asarray(bias)

    pl = build_plan(x, edge_src, edge_dst, edge_vals, weight, bias)
    nc = build_bass(pl)

    from concourse.bass_utils import run_bass_kernel_spmd

    ncores = N_CORES if _n_cores is None else _n_cores
    in_maps = []
    for ci in range(ncores):
        in_maps.append({
            "xt": pl.xT,
            "w": pl.W,
            "bvec": pl.bvec,
            "degv": np.ascontiguousarray(pl.degv[ci]),
            "iota": pl.iota,
            "idx": np.ascontiguousarray(pl.IDX[ci]),
            "dlt": np.ascontiguousarray(pl.dlt[ci]),
            "val": np.ascontiguousarray(pl.val[ci]),
        })
    res = run_bass_kernel_spmd(nc, in_maps, core_ids=list(range(ncores)),
                               trace=_want_trace)
    outs = [res.results[ci]["out"][:pl.ndst, :] for ci in range(ncores)]
    if ncores < N_CORES:
        outs += [np.zeros((pl.ndst, pl.OUT_F), np.float32)] * (
            N_CORES - ncores)
    full = np.concatenate(outs, axis=0).astype(np.float32)
    if _want_trace:
        kernel._last_results = res
    return full
